# revision 1
# baseline (speedup 1.0000x reference)
"""Triangle (starting-node) attention kernel for Trainium2, 8 NeuronCores.

Shards the I axis (rows of the pair representation) across 8 cores, weights
replicated. Each core runs LayerNorm + QKVG projections + per-row softmax
attention + gated output projection + residual on its 32 rows.

Layout strategy per core (token = (i, j) pair, 8192 tokens per core):
  - LayerNorm in natural [token, C] layout (bn_stats over free dim).
  - z transposed via PE identity-matmul to [C, token] so projections can
    contract over C.
  - q, k, g produced directly transposed [HD, token] (lhsT = W); v produced
    natural [token, HD] (lhsT = zT).
  - scores computed transposed: sT[k, q] = k . q per head, so softmax sums
    over the partition axis are done on the PE (ones-matmul) and the
    normalization is deferred: o_unnorm = v^T e, then scaled by 1/colsum
    broadcast via a tiny selector matmul, folded into the sigmoid gate.
"""

import numpy as np
import ml_dtypes
from contextlib import ExitStack

import concourse.bass as bass
import concourse.bacc as bacc
import concourse.mybir as mybir
import concourse.tile as tile
from concourse.bass_utils import run_bass_kernel_spmd
from concourse.masks import make_identity

F32 = mybir.dt.float32
BF16 = mybir.dt.bfloat16
AF = mybir.ActivationFunctionType
ALU = mybir.AluOpType

N_CORES = 8
I_FULL, J, C = 256, 256, 128
H, D = 4, 32
HD = H * D  # 128
I_LOC = I_FULL // N_CORES  # 32 rows per core
T_LOC = I_LOC * J          # 8192 tokens per core
NT = T_LOC // 128          # 64 token tiles
NG = 4                     # stat groups for batched rsqrt
GT = NT // NG              # 16 tiles per group
EPS = 1e-5

_PROG_CACHE = {}


def _build_program():
    nc = bacc.Bacc("TRN2", target_bir_lowering=False, debug=False)

    x_d = nc.dram_tensor("x", [T_LOC, C], F32, kind="ExternalInput")
    wpack_d = nc.dram_tensor("wpack", [128, 6 * 128 + 64], BF16,
                             kind="ExternalInput")
    sel_d = nc.dram_tensor("sel8", [8, 2 * 128], F32, kind="ExternalInput")
    out_d = nc.dram_tensor("out", [T_LOC, C], F32, kind="ExternalOutput")

    # token t = 128*tile + p views
    x_tiles = x_d.ap().rearrange("(g t p) c -> g p t c", p=128, t=GT)
    out_rows = out_d.ap().rearrange("(i b p) c -> i p b c", b=2, p=128)

    with tile.TileContext(nc) as tc, ExitStack() as ctx:
        singles = ctx.enter_context(tc.tile_pool(name="singles", bufs=1))
        wpack = singles.tile([128, 6 * 128 + 64], BF16)
        nc.sync.dma_start(out=wpack[:], in_=wpack_d.ap())
        w_tiles = {}
        for wi, name in enumerate(("wq", "wk", "wv", "wg", "wo", "ident")):
            w_tiles[name] = wpack[:, 128 * wi:128 * (wi + 1)]
        ident = w_tiles["ident"]
        osel_t = wpack[:, 6 * 128:6 * 128 + 64]
        eps_t = singles.tile([128, 1], F32)
        nc.vector.memset(eps_t[:], EPS)
        sel_t = singles.tile([8, 2 * 128], F32)
        nc.sync.dma_start(out=sel_t[:], in_=sel_d.ap())

        bigs = ctx.enter_context(tc.tile_pool(name="bigs", bufs=1))
        qT = bigs.tile([128, T_LOC], BF16, tag="qT")
        kT = bigs.tile([128, T_LOC], BF16, tag="kT")
        gT = bigs.tile([128, T_LOC], BF16, tag="gT")
        vb = bigs.tile([128, T_LOC], BF16, tag="vb")  # col 128*t+hd
        xb = bigs.tile([128, NT, C], F32, tag="xb")   # resident input
        zT = bigs.tile([128, T_LOC], BF16, tag="zT")
        stats_b = bigs.tile([128, NT, 6], F32, tag="stats_b")
        rbuf = bigs.tile([128, NT], F32, tag="rbuf")
        negmur = bigs.tile([128, NT], F32, tag="negmur")
        mbuf = bigs.tile([128, NT], F32, tag="mbuf")
        dbuf = bigs.tile([128, NT], F32, tag="dbuf")
        vbuf = bigs.tile([128, NT], F32, tag="vbuf")

        psS = ctx.enter_context(tc.tile_pool(name="psS", bufs=1, space="PSUM"))
        psP = ctx.enter_context(tc.tile_pool(name="psP", bufs=5, space="PSUM"))
        ep = ctx.enter_context(tc.tile_pool(name="ea", bufs=6))
        ogp = ctx.enter_context(tc.tile_pool(name="oga", bufs=4))
        outp = ctx.enter_context(tc.tile_pool(name="outa", bufs=3))
        zp = ctx.enter_context(tc.tile_pool(name="za", bufs=10))

        # ---- Stage 0: load x; LayerNorm stats via batched bn_stats ----
        # PE warmup: dependency-free matmuls so HAM is warm when the real
        # pipeline arrives (stage-0 stats otherwise leave the PE idle)
        wps = psP.tile([128, 512], F32, name="wps", tag="ps")
        for wu in range(64):
            nc.tensor.matmul(wps[:, 0:128], ident, ident,
                             start=True, stop=True)

        xhalf = x_d.ap().rearrange("(g t p) c -> g p t c", p=128, t=GT // 2)
        for gh in range(2 * NG):
            nc.sync.dma_start(
                out=xb[:, (GT // 2) * gh:(GT // 2) * (gh + 1), :],
                in_=xhalf[gh])
        sq_scr = bigs.tile([128, C], BF16, tag="sq_scr")
        for g in range(NG):
            gsl = slice(GT * g, GT * (g + 1))
            on_act = (g == NG - 1)
            if on_act:
                # ScalarE path: accumulate sum(x) and sum(x^2) per tile
                for tt in range(GT):
                    t0 = GT * g + tt
                    nc.scalar.activation(out=sq_scr[:], in_=xb[:, t0, :],
                                         func=AF.Copy,
                                         accum_out=mbuf[:, t0:t0 + 1])
                    nc.scalar.activation(out=sq_scr[:], in_=xb[:, t0, :],
                                         func=AF.Square,
                                         accum_out=vbuf[:, t0:t0 + 1])
                nc.vector.tensor_scalar_mul(mbuf[:, gsl], mbuf[:, gsl],
                                            1.0 / C)  # mean
                nc.vector.tensor_mul(dbuf[:, gsl], mbuf[:, gsl], mbuf[:, gsl])
                nc.vector.scalar_tensor_tensor(              # var
                    out=vbuf[:, gsl], in0=vbuf[:, gsl], scalar=1.0 / C,
                    in1=dbuf[:, gsl], op0=ALU.mult, op1=ALU.subtract)
            else:
                for tt in range(GT):
                    t0 = GT * g + tt
                    nc.vector.bn_stats(out=stats_b[:, t0, :],
                                       in_=xb[:, t0, :])
                s1 = stats_b[:, gsl, 1]
                s2 = stats_b[:, gsl, 2]
                s4 = stats_b[:, gsl, 4]
                s5 = stats_b[:, gsl, 5]
                nc.vector.tensor_add(mbuf[:, gsl], s1, s4)       # me + mo
                nc.vector.tensor_sub(dbuf[:, gsl], s1, s4)       # me - mo
                nc.vector.tensor_add(vbuf[:, gsl], s2, s5)       # 64*(ve+vo)
                nc.vector.scalar_tensor_tensor(                  # 0.25 d^2
                    out=dbuf[:, gsl], in0=dbuf[:, gsl], scalar=0.25,
                    in1=dbuf[:, gsl], op0=ALU.mult, op1=ALU.mult)
                nc.vector.scalar_tensor_tensor(                  # var
                    out=vbuf[:, gsl], in0=vbuf[:, gsl], scalar=1.0 / C,
                    in1=dbuf[:, gsl], op0=ALU.mult, op1=ALU.add)
                nc.vector.tensor_scalar_mul(mbuf[:, gsl], mbuf[:, gsl], 0.5)
            nc.scalar.activation(out=vbuf[:, gsl], in_=vbuf[:, gsl],
                                 func=AF.Sqrt, bias=eps_t[:], scale=1.0)
            nc.vector.reciprocal(out=rbuf[:, gsl], in_=vbuf[:, gsl])
            nc.vector.scalar_tensor_tensor(                  # -mean * r
                out=negmur[:, gsl], in0=mbuf[:, gsl], scalar=-1.0,
                in1=rbuf[:, gsl], op0=ALU.mult, op1=ALU.mult)

        # ---- Software-pipelined main loop ----
        zts = {}    # chunk -> list of 4 affine'd tiles
        eTs = {}    # row -> eT tile
        opss = {}   # row -> o psum tile
        smss = {}   # chunk -> packed [8, 256] sums psum
        rss = {}    # chunk -> [8, 256] reciprocal tile
        csps = {}   # row -> cinv broadcast psum
        ogs = {}    # row -> og tile

        def st_affine(c):
            zts[c] = []
            for tt in range(4):
                tg = 4 * c + tt
                zt = zp.tile([128, C], BF16, name="zt")
                nc.gpsimd.tensor_scalar(
                    out=zt[:], in0=xb[:, tg, :],
                    scalar1=rbuf[:, tg:tg + 1], scalar2=negmur[:, tg:tg + 1],
                    op0=ALU.mult, op1=ALU.add)
                zts[c].append(zt)

        def st_transpose(c):
            zps = psP.tile([128, 512], F32, name="zps", tag="ps")
            for tt in range(4):
                nc.tensor.matmul(zps[:, 128 * tt:128 * (tt + 1)],
                                 zts[c][tt][:], ident[:],
                                 start=True, stop=True)
            del zts[c]
            nc.vector.tensor_copy(zT[:, 512 * c:512 * (c + 1)], zps[:])

        def st_proj(c):
            sl = slice(512 * c, 512 * (c + 1))
            for wname, dst in (("wq", qT), ("wk", kT)):
                ps = psP.tile([128, 512], F32, name="psq", tag="ps")
                nc.tensor.matmul(ps[:], w_tiles[wname][:], zT[:, sl],
                                 start=True, stop=True)
                nc.scalar.copy(dst[:, sl], ps[:])
            ps = psP.tile([128, 512], F32, name="psg", tag="ps")
            nc.tensor.matmul(ps[:], w_tiles["wg"][:], zT[:, sl],
                             start=True, stop=True)
            nc.scalar.activation(out=gT[:, sl], in_=ps[:],
                                 func=AF.Tanh, bias=0.0, scale=0.5)
            psv = psP.tile([128, 512], F32, name="psv", tag="ps")
            for tt in range(4):
                t4 = 4 * c + tt
                nc.tensor.matmul(psv[:, 128 * tt:128 * (tt + 1)],
                                 zT[:, 128 * t4:128 * (t4 + 1)],
                                 w_tiles["wv"][:], start=True, stop=True)
            nc.vector.tensor_copy(vb[:, sl], psv[:])

        def st_scores(i, p):
            # scores pair p of row i, transposed, + exp
            if p == 0:
                eTs[i] = ep.tile([128, 2048], BF16, name="eT")
            tsl = slice(256 * i, 256 * (i + 1))
            sps = psS.tile([128, 1024], F32, name="sps", tag="sps")
            for hh in range(2):
                h = 2 * p + hh
                hsl = slice(32 * h, 32 * (h + 1))
                for kb in range(2):
                    nc.tensor.matmul(
                        sps[:, 512 * hh + 256 * kb:512 * hh + 256 * (kb + 1)],
                        kT[hsl, 256 * i + 128 * kb:256 * i + 128 * (kb + 1)],
                        qT[hsl, tsl],
                        start=True, stop=True,
                        tile_position=(32 * h, 0))
            nc.scalar.activation(out=eTs[i][:, 1024 * p:1024 * (p + 1)],
                                 in_=sps[:], func=AF.Exp, bias=0.0, scale=1.0)

        def st_osums(j, rp):
            # o and packed col-sums for row i = 2j + rp
            i = 2 * j + rp
            if rp == 0:
                smss[j] = psS.tile([8, 512], F32, name="sms", tag="sms")
            ops = psP.tile([128, 256], F32, name="ops", tag="ps")
            opss[i] = ops
            eT = eTs[i]
            for h in range(H):
                p, hh = divmod(h, 2)
                for kb in range(2):
                    esl = slice(1024 * p + 512 * hh + 256 * kb,
                                1024 * p + 512 * hh + 256 * (kb + 1))
                    vt = 2 * i + kb
                    nc.tensor.matmul(
                        ops[32 * h:32 * (h + 1), :],
                        vb[:, 128 * vt + 32 * h:128 * vt + 32 * (h + 1)],
                        eT[:, esl],
                        start=(kb == 0), stop=(kb == 1),
                        tile_position=(0, 32 * h))
                if True:
                    jj = 4 * rp + h
                    hesl = slice(1024 * p + 512 * hh, 1024 * p + 512 * (hh + 1))
                    nc.tensor.matmul(
                        smss[j][:], osel_t[:, 8 * jj:8 * (jj + 1)], eT[:, hesl],
                        start=(rp == 0 and h == 0),
                        stop=(rp == 1 and h == 3),
                        tile_position=(0, 0))
            if rp == 1:
                del eTs[2 * j], eTs[2 * j + 1]

        rsbs = {}

        def st_recip(j):
            # reciprocal of both rows' (doubled) sums + broadcast via DMA
            ssum = ogp.tile([8, 256], F32, tag="ssum", name="ssum")
            # sum the two kb halves: view [8, q, kb] and reduce innermost
            sview = smss[j].rearrange("p (kb q) -> p q kb", kb=2)
            nc.vector.tensor_reduce(out=ssum[:], in_=sview,
                                    axis=mybir.AxisListType.X,
                                    op=ALU.add)
            del smss[j]
            rs = ogp.tile([8, 256], F32, tag="rs", name="rs")
            rscr = ogp.tile([8, 256], F32, tag="rscr", name="rscr")
            nc.vector.reciprocal_approx_accurate(out=rs[:], in_=ssum[:],
                                                 scratch=rscr[:])
            for rp in range(2):
                i = 2 * j + rp
                rsb = psP.tile([128, 256], F32, tag="ps", name="rsb")
                rsbs[i] = rsb
                nc.tensor.matmul(rsb[:], sel_t[:, 128 * rp:128 * (rp + 1)],
                                 rs[:], start=True, stop=True)

        def st_gate(j):
            for rp in range(2):
                i = 2 * j + rp
                tsl = slice(256 * i, 256 * (i + 1))
                gc = ogp.tile([128, 256], F32, tag="gc", name="gc")
                nc.vector.scalar_tensor_tensor(
                    out=gc[:], in0=gT[:, tsl], scalar=1.0, in1=rsbs[i][:],
                    op0=ALU.add, op1=ALU.mult)
                del rsbs[i]
                og = ogp.tile([128, 256], BF16, tag="og", name="og")
                ogs[i] = og
                nc.vector.tensor_mul(og[:], gc[:], opss[i][:])
                del opss[i]

        def st_out(j):
            for rp in range(2):
                i = 2 * j + rp
                psy = psP.tile([128, 2, 128], F32, name="psy", tag="ps")
                for qb in range(2):
                    nc.tensor.matmul(psy[:, qb, :],
                                     ogs[i][:, 128 * qb:128 * (qb + 1)],
                                     w_tiles["wo"][:], start=True, stop=True)
                del ogs[i]
                ot = outp.tile([128, 2, 128], F32, name="ot")
                nc.vector.tensor_add(ot[:], xb[:, 2 * i:2 * (i + 1), :],
                                     psy[:])
                nc.sync.dma_start(out=out_rows[i], in_=ot[:])

        NCH = T_LOC // 512  # 16 chunks of 512 tokens
        for it in range(NCH + 6):
            j5, j4, j3 = it - 5, it - 4, it - 3
            c2, c1, c0 = it - 2, it - 1, it
            if 0 <= j5 < NCH:
                st_out(j5)
            if 0 <= j3 < NCH:
                st_scores(2 * j3, 0)
            if 0 <= j4 < NCH:
                st_osums(j4, 0)
            if 0 <= j3 < NCH:
                st_scores(2 * j3, 1)
            if 0 <= j4 < NCH:
                st_osums(j4, 1)
                st_recip(j4)
            if 0 <= j3 < NCH:
                st_scores(2 * j3 + 1, 0)
            if 0 <= c1 < NCH:
                st_transpose(c1)
            if 0 <= j3 < NCH:
                st_scores(2 * j3 + 1, 1)
            if 0 <= c2 < NCH:
                st_proj(c2)
            if 0 <= j4 < NCH:
                st_gate(j4)
            if 0 <= c0 < NCH:
                st_affine(c0)
            if it < 3:  # keep PE warm through pipeline fill
                for wu in range(12):
                    nc.tensor.matmul(wps[:, 0:128], ident, ident,
                                     start=True, stop=True)

    nc.compile()
    return nc


def _get_program():
    key = "v1"
    if key not in _PROG_CACHE:
        _PROG_CACHE[key] = _build_program()
    return _PROG_CACHE[key]


def _prepare_in_maps(inputs):
    x = np.asarray(inputs["x"], dtype=np.float32)
    mask = np.asarray(inputs["mask"])
    ln_g = np.asarray(inputs["ln_g"], dtype=np.float32)
    ln_b = np.asarray(inputs["ln_b"], dtype=np.float32)
    Wq = np.asarray(inputs["Wq"], dtype=np.float32)
    Wk = np.asarray(inputs["Wk"], dtype=np.float32)
    Wv = np.asarray(inputs["Wv"], dtype=np.float32)
    Wg = np.asarray(inputs["Wg"], dtype=np.float32)
    bg = np.asarray(inputs["bg"], dtype=np.float32)
    Wo = np.asarray(inputs["Wo"], dtype=np.float32)
    bo = np.asarray(inputs["bo"], dtype=np.float32)

    assert bool(mask.all()), "kernel currently requires an all-True mask"
    assert np.all(ln_b == 0.0) and np.all(bg == 0.0), \
        "kernel currently requires zero ln_b/bg biases"

    scale = 1.0 / np.sqrt(np.float32(D))
    bf = ml_dtypes.bfloat16
    wq = ((ln_g[:, None] * Wq) * scale).astype(bf)
    wk = (ln_g[:, None] * Wk).astype(bf)
    wv = (ln_g[:, None] * Wv).astype(bf)
    wg = (ln_g[:, None] * Wg).astype(bf)

    # sel8[:, 128*rp + m] = 0.5 iff r == 4*rp + m//32 (0.5 folds sigmoid)
    sel = np.zeros((8, 2 * 128), dtype=np.float32)
    for rp in range(2):
        for h in range(H):
            sel[4 * rp + h, 128 * rp + 32 * h:128 * rp + 32 * (h + 1)] = 1.0
    # onesel block jj: [128, 8] with column jj all ones
    osel = np.zeros((128, 64), dtype=ml_dtypes.bfloat16)
    for jj in range(8):
        osel[:, 8 * jj + jj] = 2.0  # doubled: recip then gives 0.5/sum

    xr = (x + bo).astype(np.float32)  # residual folds the output bias
    B = x.shape[0]
    assert B == 1 and x.shape[1] == I_FULL

    wpack = np.concatenate(
        [wq, wk, wv, wg, Wo.astype(bf), np.eye(128, dtype=bf), osel], axis=1)
    wpack = np.ascontiguousarray(wpack)

    in_maps = []
    for c in range(N_CORES):
        xs = np.ascontiguousarray(
            xr[0, I_LOC * c:I_LOC * (c + 1)].reshape(T_LOC, C))
        in_maps.append({"x": xs, "wpack": wpack, "sel8": sel})
    return in_maps


def run_sharded(inputs, trace=False, **kw):
    nc = _get_program()
    in_maps = _prepare_in_maps(inputs)
    res = run_bass_kernel_spmd(nc, in_maps, core_ids=list(range(N_CORES)),
                               trace=trace, **kw)
    shards = [res.results[c]["out"].reshape(1, I_LOC, J, C)
              for c in range(N_CORES)]
    out = np.concatenate(shards, axis=1)
    return out, res


def kernel(**inputs) -> np.ndarray:
    out, _ = run_sharded(inputs, trace=False)
    return out



# revision 9
# speedup vs baseline: 1.0392x; 1.0392x over previous
"""Triangle (starting-node) attention kernel for Trainium2, 8 NeuronCores.

Shards the I axis (rows of the pair representation) across 8 cores, weights
replicated. Each core runs LayerNorm + QKVG projections + per-row softmax
attention + gated output projection + residual on its 32 rows.

Layout strategy per core (token = (i, j) pair, 8192 tokens per core):
  - LayerNorm stats via bn_stats (DVE); affine on GPSIMD in natural layout.
  - z transposed via PE identity-matmul to [C, token] so projections contract
    over C.
  - q, k produced transposed [HD, token]; g = sigmoid(z Wg) transposed;
    v natural [token, HD] (vb col layout 128*t + hd).
  - scores transposed sT[k, q] per head, all 4 heads concurrently on the 4
    PE row-group strips; exp in a single [128, 2048] ACT call per row.
  - o = v^T e on the 4 col-group strips; softmax denominators via ones-weight
    matmuls on the same strips accumulating both key blocks and both rows of
    a pair into one partition-coded [128, 256] PSUM tile.
  - reciprocal_approx_fast per pair, broadcast to [HD, tok] via fp32r
    selector matmuls, gate/normalize on DVE.
  - output projection accumulates og @ Wo on top of an identity-matmul
    residual (fp32r, N=512), DMA to HBM straight from PSUM.
"""

import numpy as np
import ml_dtypes
from contextlib import ExitStack

import concourse.bass as bass
import concourse.bacc as bacc
import concourse.mybir as mybir
import concourse.tile as tile
from concourse.bass_utils import run_bass_kernel_spmd

F32 = mybir.dt.float32
F32R = mybir.dt.float32r
BF16 = mybir.dt.bfloat16
AF = mybir.ActivationFunctionType
ALU = mybir.AluOpType

N_CORES = 8
I_FULL, J, C = 256, 256, 128
H, D = 4, 32
HD = H * D  # 128
I_LOC = I_FULL // N_CORES  # 32 rows per core
T_LOC = I_LOC * J          # 8192 tokens per core
NT = T_LOC // 128          # 64 token tiles
NG = 4                     # stat groups for batched rsqrt
GT = NT // NG              # 16 tiles per group
NCH = T_LOC // 512         # 16 chunks of 512 tokens (= row pairs)
EPS = 1e-5

_PROG_CACHE = {}


def _build_program():
    nc = bacc.Bacc("TRN2", target_bir_lowering=False, debug=False)

    x_d = nc.dram_tensor("x", [T_LOC, C], F32, kind="ExternalInput")
    wpack_d = nc.dram_tensor("wpack", [128, 6 * 128 + 64], BF16,
                             kind="ExternalInput")
    sel_d = nc.dram_tensor("selpack", [128, 2 * 128], BF16,
                           kind="ExternalInput")
    out_d = nc.dram_tensor("out", [T_LOC, C], F32, kind="ExternalOutput")

    xhalf = x_d.ap().rearrange("(g t p) c -> g p t c", p=128, t=GT // 2)
    out_pairs = out_d.ap().rearrange("(j b p) c -> j p b c", b=4, p=128)

    with tile.TileContext(nc) as tc, ExitStack() as ctx:
        singles = ctx.enter_context(tc.tile_pool(name="singles", bufs=1))
        wpack = singles.tile([128, 6 * 128 + 64], BF16)
        nc.sync.dma_start(out=wpack[:], in_=wpack_d.ap())
        w_tiles = {}
        for wi, name in enumerate(("wq", "wk", "wv", "wg", "wo", "ident")):
            w_tiles[name] = wpack[:, 128 * wi:128 * (wi + 1)]
        ident = w_tiles["ident"]
        ones_rp = [wpack[:, 6 * 128:6 * 128 + 32],
                   wpack[:, 6 * 128 + 32:6 * 128 + 64]]
        sel_t = singles.tile([128, 2 * 128], BF16)
        nc.sync.dma_start(out=sel_t[:], in_=sel_d.ap())
        eps_t = singles.tile([128, 1], F32)
        nc.vector.memset(eps_t[:], EPS)

        bigs = ctx.enter_context(tc.tile_pool(name="bigs", bufs=1))
        qT = bigs.tile([128, T_LOC], BF16, tag="qT")
        kT = bigs.tile([128, T_LOC], BF16, tag="kT")
        gT = bigs.tile([128, T_LOC], BF16, tag="gT")
        vb = bigs.tile([128, T_LOC], BF16, tag="vb")  # col 128*t+hd
        zT = bigs.tile([128, T_LOC], BF16, tag="zT")
        xb = bigs.tile([128, NT, C], F32, tag="xb")   # resident input
        stats_b = bigs.tile([128, NT, 6], F32, tag="stats_b")
        rbuf = bigs.tile([128, NT], F32, tag="rbuf")
        negmur = bigs.tile([128, NT], F32, tag="negmur")
        mbuf = bigs.tile([128, NT], F32, tag="mbuf")
        dbuf = bigs.tile([128, NT], F32, tag="dbuf")
        vbuf = bigs.tile([128, NT], F32, tag="vbuf")

        # PSUM: 8 banks total.
        #   sps  : 1 x [128,2048] f32 = 4 banks (scores -> exp)
        #   smz  : 1 x [128, 512]      = 1 bank  ({sms colsum, zps transpose})
        #   opg  : 1 x [128, 512]      = 1 bank  ({ops o-accum, psg g-proj})
        #   pp   : 2 x [128, 512]      = 2 banks ({psy, rsb, psq, psk, psv})
        spsP = ctx.enter_context(tc.tile_pool(name="spsP", bufs=1,
                                              space="PSUM"))
        smzP = ctx.enter_context(tc.tile_pool(name="smzP", bufs=1,
                                              space="PSUM"))
        opgP = ctx.enter_context(tc.tile_pool(name="opgP", bufs=1,
                                              space="PSUM"))
        ppP = ctx.enter_context(tc.tile_pool(name="ppP", bufs=2, space="PSUM"))

        ep = ctx.enter_context(tc.tile_pool(name="ea", bufs=4))
        rsp = ctx.enter_context(tc.tile_pool(name="rsa", bufs=2))
        gcp = ctx.enter_context(tc.tile_pool(name="gca", bufs=2))
        ogp = ctx.enter_context(tc.tile_pool(name="oga", bufs=2))
        outp = ctx.enter_context(tc.tile_pool(name="outa", bufs=2))
        zp = ctx.enter_context(tc.tile_pool(name="za", bufs=10))

        # ---- Stage 0: load x; LayerNorm stats via batched bn_stats ----
        # PE warmup: dependency-free matmuls keep HAM warm until the real
        # pipeline arrives.
        wps = ppP.tile([128, 512], F32, name="wps", tag="pp")
        for wu in range(64):
            nc.tensor.matmul(wps[:, 0:128], ident, ident,
                             start=True, stop=True)

        for gh in range(2 * NG):
            nc.sync.dma_start(
                out=xb[:, (GT // 2) * gh:(GT // 2) * (gh + 1), :],
                in_=xhalf[gh])
        for g in range(NG):
            gsl = slice(GT * g, GT * (g + 1))
            for tt in range(GT):
                t0 = GT * g + tt
                nc.vector.bn_stats(out=stats_b[:, t0, :], in_=xb[:, t0, :])
            s1 = stats_b[:, gsl, 1]
            s2 = stats_b[:, gsl, 2]
            s4 = stats_b[:, gsl, 4]
            s5 = stats_b[:, gsl, 5]
            nc.vector.tensor_add(mbuf[:, gsl], s1, s4)       # me + mo
            nc.vector.tensor_sub(dbuf[:, gsl], s1, s4)       # me - mo
            nc.vector.tensor_add(vbuf[:, gsl], s2, s5)       # 64*(ve+vo)
            nc.vector.scalar_tensor_tensor(                  # 0.25 d^2
                out=dbuf[:, gsl], in0=dbuf[:, gsl], scalar=0.25,
                in1=dbuf[:, gsl], op0=ALU.mult, op1=ALU.mult)
            nc.vector.scalar_tensor_tensor(                  # var
                out=vbuf[:, gsl], in0=vbuf[:, gsl], scalar=1.0 / C,
                in1=dbuf[:, gsl], op0=ALU.mult, op1=ALU.add)
            nc.vector.tensor_scalar_mul(mbuf[:, gsl], mbuf[:, gsl], 0.5)
            nc.scalar.activation(out=vbuf[:, gsl], in_=vbuf[:, gsl],
                                 func=AF.Sqrt, bias=eps_t[:], scale=1.0)
            nc.vector.reciprocal(out=rbuf[:, gsl], in_=vbuf[:, gsl])
            nc.vector.scalar_tensor_tensor(                  # -mean * r
                out=negmur[:, gsl], in0=mbuf[:, gsl], scalar=-1.0,
                in1=rbuf[:, gsl], op0=ALU.mult, op1=ALU.mult)

        # ---- Software-pipelined main loop ----
        zts = {}    # chunk -> list of 4 affine'd tiles
        eTs = {}    # row -> eT tile
        opss = {}   # pair -> o psum tile [128, 512]
        smss = {}   # pair -> colsum psum tile [128, 256]
        rss = {}    # pair -> reciprocal tile [128, 256]
        rsbs = {}   # pair -> broadcast recips psum [128, 512]
        gcs = {}    # pair -> gate*recip tile
        ogs = {}    # pair -> gated o tile

        def st_affine(c):
            zts[c] = []
            for tt in range(4):
                tg = 4 * c + tt
                zt = zp.tile([128, C], BF16, name="zt")
                nc.gpsimd.tensor_scalar(
                    out=zt[:], in0=xb[:, tg, :],
                    scalar1=rbuf[:, tg:tg + 1], scalar2=negmur[:, tg:tg + 1],
                    op0=ALU.mult, op1=ALU.add)
                zts[c].append(zt)

        def st_transpose(c):
            zps = smzP.tile([128, 512], F32, name="zps", tag="smz")
            for tt in range(4):
                nc.tensor.matmul(zps[:, 128 * tt:128 * (tt + 1)],
                                 zts[c][tt][:], ident[:],
                                 start=True, stop=True)
            del zts[c]
            nc.vector.tensor_copy(zT[:, 512 * c:512 * (c + 1)], zps[:])

        def st_proj(c):
            sl = slice(512 * c, 512 * (c + 1))
            for wname, dst in (("wq", qT), ("wk", kT)):
                ps = ppP.tile([128, 512], F32, name="psq", tag="pp")
                nc.tensor.matmul(ps[:], w_tiles[wname][:], zT[:, sl],
                                 start=True, stop=True)
                nc.vector.tensor_copy(dst[:, sl], ps[:])
            psg = opgP.tile([128, 512], F32, name="psg", tag="opg")
            nc.tensor.matmul(psg[:], w_tiles["wg"][:], zT[:, sl],
                             start=True, stop=True)
            nc.scalar.activation(out=gT[:, sl], in_=psg[:],
                                 func=AF.Sigmoid, bias=0.0, scale=1.0)
            psv = ppP.tile([128, 512], F32, name="psv", tag="pp")
            for tt in range(4):
                t4 = 4 * c + tt
                nc.tensor.matmul(psv[:, 128 * tt:128 * (tt + 1)],
                                 zT[:, 128 * t4:128 * (t4 + 1)],
                                 w_tiles["wv"][:], start=True, stop=True)
            nc.vector.tensor_copy(vb[:, sl], psv[:])

        def st_scores(i):
            # scores for row i, transposed: sps[key, 512h + 256kb + q]
            sps = spsP.tile([128, 2048], F32, name="sps", tag="sps")
            for h in range(H):
                hsl = slice(32 * h, 32 * (h + 1))
                for kb in range(2):
                    nc.tensor.matmul(
                        sps[:, 512 * h + 256 * kb:512 * h + 256 * (kb + 1)],
                        kT[hsl, 256 * i + 128 * kb:256 * i + 128 * (kb + 1)],
                        qT[hsl, 256 * i:256 * (i + 1)],
                        start=True, stop=True,
                        tile_position=(32 * h, 0))
            eT = ep.tile([128, 2048], BF16, name="eT")
            eTs[i] = eT
            nc.scalar.activation(out=eT[:], in_=sps[:],
                                 func=AF.Exp, bias=0.0, scale=1.0)

        def st_osum(i):
            # o and colsums for row i = 2j + rp
            j, rp = divmod(i, 2)
            if rp == 0:
                opss[j] = opgP.tile([128, 512], F32, name="ops", tag="opg")
                smss[j] = smzP.tile([128, 256], F32, name="sms", tag="smz")
            ops, sms = opss[j], smss[j]
            eT = eTs.pop(i)
            for h in range(H):
                for kb in range(2):
                    esl = slice(512 * h + 256 * kb, 512 * h + 256 * (kb + 1))
                    vt = 2 * i + kb
                    nc.tensor.matmul(
                        ops[32 * h:32 * (h + 1), 256 * rp:256 * (rp + 1)],
                        vb[:, 128 * vt + 32 * h:128 * vt + 32 * (h + 1)],
                        eT[:, esl],
                        start=(kb == 0), stop=(kb == 1),
                        tile_position=(0, 32 * h))
                for kb in range(2):
                    esl = slice(512 * h + 256 * kb, 512 * h + 256 * (kb + 1))
                    nc.tensor.matmul(
                        sms[32 * h:32 * (h + 1), :],
                        ones_rp[rp][:], eT[:, esl],
                        start=(rp == 0 and kb == 0),
                        stop=(rp == 1 and kb == 1),
                        tile_position=(0, 32 * h))

        def st_recip(j):
            rs = rsp.tile([128, 256], F32, name="rs", tag="rs")
            nc.vector.reciprocal_approx_fast(out=rs[:], in_=smss.pop(j)[:])
            rs_bf = rsp.tile([128, 256], BF16, name="rsbf", tag="rsbf")
            rss[j] = rs_bf
            nc.gpsimd.tensor_copy(rs_bf[:], rs[:])

        def st_rsb(j):
            rsb = ppP.tile([128, 512], F32, name="rsb", tag="pp")
            rsbs[j] = rsb
            rs_bf = rss.pop(j)
            for rp in range(2):
                nc.tensor.matmul(
                    rsb[:, 256 * rp:256 * (rp + 1)],
                    sel_t[:, 128 * rp:128 * (rp + 1)],
                    rs_bf[:],
                    start=True, stop=True)

        def st_gate(j):
            sl = slice(512 * j, 512 * (j + 1))
            gc = gcp.tile([128, 512], F32, name="gc", tag="gc")
            gcs[j] = gc
            nc.vector.tensor_mul(gc[:], gT[:, sl], rsbs.pop(j)[:])
            og = ogp.tile([128, 512], BF16, name="og", tag="og")
            ogs[j] = og
            nc.vector.tensor_mul(og[:], gcs.pop(j)[:], opss.pop(j)[:])

        def st_out(j):
            psy = ppP.tile([128, 4, 128], F32, name="psy", tag="pp")
            og = ogs.pop(j)
            for b in range(4):
                nc.tensor.matmul(psy[:, b, :],
                                 og[:, 128 * b:128 * (b + 1)],
                                 w_tiles["wo"][:], start=True, stop=True)
            ot = outp.tile([128, 4, 128], F32, name="ot")
            nc.vector.tensor_add(ot[:], xb[:, 4 * j:4 * (j + 1), :], psy[:])
            nc.sync.dma_start(out=out_pairs[j], in_=ot[:])

        for it in range(NCH + 6):
            j5, j4, j3 = it - 5, it - 4, it - 3
            c2, c1, c0 = it - 2, it - 1, it
            if 0 <= j5 < NCH:
                st_out(j5)
            if 0 <= j3 < NCH:
                st_scores(2 * j3)
            if 0 <= j4 < NCH:
                st_osum(2 * j4)
            if 0 <= j3 < NCH:
                st_scores(2 * j3 + 1)
            if 0 <= j4 < NCH:
                st_osum(2 * j4 + 1)
                st_recip(j4)
                st_rsb(j4)
                st_gate(j4)
            if 0 <= c1 < NCH:
                st_transpose(c1)
            if 0 <= c2 < NCH:
                st_proj(c2)
            if 0 <= c0 < NCH:
                st_affine(c0)
            if it < 3:  # keep PE warm through pipeline fill
                for wu in range(12):
                    nc.tensor.matmul(wps[:, 0:128], ident, ident,
                                     start=True, stop=True)

    nc.compile()
    return nc


def _get_program():
    key = "v2"
    if key not in _PROG_CACHE:
        _PROG_CACHE[key] = _build_program()
    return _PROG_CACHE[key]


def _prepare_in_maps(inputs):
    x = np.asarray(inputs["x"], dtype=np.float32)
    mask = np.asarray(inputs["mask"])
    ln_g = np.asarray(inputs["ln_g"], dtype=np.float32)
    ln_b = np.asarray(inputs["ln_b"], dtype=np.float32)
    Wq = np.asarray(inputs["Wq"], dtype=np.float32)
    Wk = np.asarray(inputs["Wk"], dtype=np.float32)
    Wv = np.asarray(inputs["Wv"], dtype=np.float32)
    Wg = np.asarray(inputs["Wg"], dtype=np.float32)
    bg = np.asarray(inputs["bg"], dtype=np.float32)
    Wo = np.asarray(inputs["Wo"], dtype=np.float32)
    bo = np.asarray(inputs["bo"], dtype=np.float32)

    assert bool(mask.all()), "kernel currently requires an all-True mask"
    assert np.all(ln_b == 0.0) and np.all(bg == 0.0), \
        "kernel currently requires zero ln_b/bg biases"

    scale = 1.0 / np.sqrt(np.float32(D))
    bf = ml_dtypes.bfloat16
    wq = ((ln_g[:, None] * Wq) * scale).astype(bf)
    wk = (ln_g[:, None] * Wk).astype(bf)
    wv = (ln_g[:, None] * Wv).astype(bf)
    wg = (ln_g[:, None] * Wg).astype(bf)

    # colsum selectors: ones_rp0 puts row 0's sums at partition 32h+{0,2..31},
    # ones_rp1 puts row 1's sums at partition 32h+1 (no partition left zero,
    # so reciprocal_approx_fast never sees 0).
    ones0 = np.ones((128, 32), dtype=bf)
    ones0[:, 1] = 0
    ones1 = np.zeros((128, 32), dtype=bf)
    ones1[:, 1] = 1

    # selpack: sel_rp[p, m] = 1 iff p == 32*(m//32) + rp (broadcast recips).
    sel = np.zeros((128, 2 * 128), dtype=bf)
    for rp in range(2):
        for h in range(H):
            sel[32 * h + rp, 128 * rp + 32 * h:128 * rp + 32 * (h + 1)] = 1.0

    xr = (x + bo).astype(np.float32)  # residual folds the output bias
    B = x.shape[0]
    assert B == 1 and x.shape[1] == I_FULL

    wpack = np.concatenate(
        [wq, wk, wv, wg, Wo.astype(bf), np.eye(128, dtype=bf), ones0, ones1],
        axis=1)
    wpack = np.ascontiguousarray(wpack)

    in_maps = []
    for c in range(N_CORES):
        xs = np.ascontiguousarray(
            xr[0, I_LOC * c:I_LOC * (c + 1)].reshape(T_LOC, C))
        in_maps.append({"x": xs, "wpack": wpack, "selpack": sel})
    return in_maps


def run_sharded(inputs, trace=False, **kw):
    nc = _get_program()
    in_maps = _prepare_in_maps(inputs)
    res = run_bass_kernel_spmd(nc, in_maps, core_ids=list(range(N_CORES)),
                               trace=trace, **kw)
    shards = [res.results[c]["out"].reshape(1, I_LOC, J, C)
              for c in range(N_CORES)]
    out = np.concatenate(shards, axis=1)
    return out, res


def kernel(**inputs) -> np.ndarray:
    out, _ = run_sharded(inputs, trace=False)
    return out


# revision 12
# speedup vs baseline: 1.1085x; 1.0667x over previous
"""Triangle (starting-node) attention kernel for Trainium2, 8 NeuronCores.

Shards the I axis (rows of the pair representation) across 8 cores, weights
replicated. Each core runs LayerNorm + QKVG projections + per-row softmax
attention + gated output projection + residual on its 32 rows.

Layout strategy per core (token = (i, j) pair, 8192 tokens per core):
  - LayerNorm stats via bn_stats (DVE); affine on GPSIMD in natural layout.
  - z transposed via PE identity-matmul to [C, token] so projections contract
    over C.
  - q, k produced transposed [HD, token]; g = sigmoid(z Wg) transposed;
    v natural [token, HD] (vb col layout 128*t + hd).
  - scores transposed sT[k, q] per head, all 4 heads concurrently on the 4
    PE row-group strips; exp in a single [128, 2048] ACT call per row.
  - o = v^T e on the 4 col-group strips; softmax denominators via ones-weight
    matmuls on the same strips accumulating both key blocks and both rows of
    a pair into one partition-coded [128, 256] PSUM tile.
  - reciprocal_approx_fast per pair, broadcast to [HD, tok] via fp32r
    selector matmuls, gate/normalize on DVE.
  - output projection accumulates og @ Wo on top of an identity-matmul
    residual (fp32r, N=512), DMA to HBM straight from PSUM.
"""

import numpy as np
import ml_dtypes
from contextlib import ExitStack

import concourse.bass as bass
import concourse.bacc as bacc
import concourse.mybir as mybir
import concourse.tile as tile
from concourse.bass_utils import run_bass_kernel_spmd

F32 = mybir.dt.float32
F32R = mybir.dt.float32r
BF16 = mybir.dt.bfloat16
AF = mybir.ActivationFunctionType
ALU = mybir.AluOpType

N_CORES = 8
I_FULL, J, C = 256, 256, 128
H, D = 4, 32
HD = H * D  # 128
I_LOC = I_FULL // N_CORES  # 32 rows per core
T_LOC = I_LOC * J          # 8192 tokens per core
NT = T_LOC // 128          # 64 token tiles
NG = 4                     # stat groups for batched rsqrt
GT = NT // NG              # 16 tiles per group
NCH = T_LOC // 512         # 16 chunks of 512 tokens (= row pairs)
EPS = 1e-5

_PROG_CACHE = {}


def _build_program():
    nc = bacc.Bacc("TRN2", target_bir_lowering=False, debug=False)

    x_d = nc.dram_tensor("x", [T_LOC, C], F32, kind="ExternalInput")
    wpack_d = nc.dram_tensor("wpack", [128, 6 * 128 + 64], BF16,
                             kind="ExternalInput")
    sel_d = nc.dram_tensor("selpack", [128, 2 * 128], BF16,
                           kind="ExternalInput")
    out_d = nc.dram_tensor("out", [T_LOC, C], F32, kind="ExternalOutput")

    xhalf = x_d.ap().rearrange("(g t p) c -> g p t c", p=128, t=GT // 2)
    out_pairs = out_d.ap().rearrange("(j b p) c -> j p b c", b=4, p=128)

    with tile.TileContext(nc) as tc, ExitStack() as ctx:
        singles = ctx.enter_context(tc.tile_pool(name="singles", bufs=1))
        wpack = singles.tile([128, 6 * 128 + 64], BF16)
        nc.sync.dma_start(out=wpack[:], in_=wpack_d.ap())
        w_tiles = {}
        for wi, name in enumerate(("wq", "wk", "wv", "wg", "wo", "ident")):
            w_tiles[name] = wpack[:, 128 * wi:128 * (wi + 1)]
        ident = w_tiles["ident"]
        ones_rp = [wpack[:, 6 * 128:6 * 128 + 32],
                   wpack[:, 6 * 128 + 32:6 * 128 + 64]]
        sel_t = singles.tile([128, 2 * 128], BF16)
        nc.sync.dma_start(out=sel_t[:], in_=sel_d.ap())
        eps_t = singles.tile([128, 1], F32)
        nc.vector.memset(eps_t[:], EPS)

        bigs = ctx.enter_context(tc.tile_pool(name="bigs", bufs=1))
        qT = bigs.tile([128, T_LOC], BF16, tag="qT")
        kT = bigs.tile([128, T_LOC], BF16, tag="kT")
        gT = bigs.tile([128, T_LOC], BF16, tag="gT")
        vb = bigs.tile([128, T_LOC], BF16, tag="vb")  # col 128*t+hd
        zT = bigs.tile([128, T_LOC], BF16, tag="zT")
        xb = bigs.tile([128, NT, C], F32, tag="xb")   # resident input
        stats_b = bigs.tile([128, NT, 6], F32, tag="stats_b")
        rbuf = bigs.tile([128, NT], F32, tag="rbuf")
        negmur = bigs.tile([128, NT], F32, tag="negmur")
        mbuf = bigs.tile([128, NT], F32, tag="mbuf")
        dbuf = bigs.tile([128, NT], F32, tag="dbuf")
        vbuf = bigs.tile([128, NT], F32, tag="vbuf")

        # PSUM: 8 banks total.
        #   sps  : 1 x [128,2048] f32 = 4 banks (scores -> exp)
        #   smz  : 1 x [128, 512]      = 1 bank  ({sms colsum, zps transpose})
        #   opg  : 1 x [128, 512]      = 1 bank  ({ops o-accum, psg g-proj})
        #   pp   : 2 x [128, 512]      = 2 banks ({psy, rsb, psq, psk, psv})
        spsP = ctx.enter_context(tc.tile_pool(name="spsP", bufs=1,
                                              space="PSUM"))
        smzP = ctx.enter_context(tc.tile_pool(name="smzP", bufs=1,
                                              space="PSUM"))
        opgP = ctx.enter_context(tc.tile_pool(name="opgP", bufs=1,
                                              space="PSUM"))
        ppP = ctx.enter_context(tc.tile_pool(name="ppP", bufs=2, space="PSUM"))

        ep = ctx.enter_context(tc.tile_pool(name="ea", bufs=4))
        rsp = ctx.enter_context(tc.tile_pool(name="rsa", bufs=2))
        gcp = ctx.enter_context(tc.tile_pool(name="gca", bufs=2))
        ogp = ctx.enter_context(tc.tile_pool(name="oga", bufs=2))
        outp = ctx.enter_context(tc.tile_pool(name="outa", bufs=2))
        zp = ctx.enter_context(tc.tile_pool(name="za", bufs=10))

        # ---- Stage 0: load x; LayerNorm stats via batched bn_stats ----
        # PE warmup: dependency-free matmuls keep HAM warm until the real
        # pipeline arrives.
        wps = ppP.tile([128, 512], F32, name="wps", tag="pp")
        for wu in range(64):
            nc.tensor.matmul(wps[:, 0:128], ident, ident,
                             start=True, stop=True)

        for gh in range(2 * NG):
            nc.sync.dma_start(
                out=xb[:, (GT // 2) * gh:(GT // 2) * (gh + 1), :],
                in_=xhalf[gh])
        for g in range(NG):
            gsl = slice(GT * g, GT * (g + 1))
            for tt in range(GT):
                t0 = GT * g + tt
                nc.vector.bn_stats(out=stats_b[:, t0, :], in_=xb[:, t0, :])
            s1 = stats_b[:, gsl, 1]
            s2 = stats_b[:, gsl, 2]
            s4 = stats_b[:, gsl, 4]
            s5 = stats_b[:, gsl, 5]
            nc.vector.tensor_add(mbuf[:, gsl], s1, s4)       # me + mo
            nc.vector.tensor_sub(dbuf[:, gsl], s1, s4)       # me - mo
            nc.vector.tensor_add(vbuf[:, gsl], s2, s5)       # 64*(ve+vo)
            nc.vector.scalar_tensor_tensor(                  # 0.25 d^2
                out=dbuf[:, gsl], in0=dbuf[:, gsl], scalar=0.25,
                in1=dbuf[:, gsl], op0=ALU.mult, op1=ALU.mult)
            nc.vector.scalar_tensor_tensor(                  # var
                out=vbuf[:, gsl], in0=vbuf[:, gsl], scalar=1.0 / C,
                in1=dbuf[:, gsl], op0=ALU.mult, op1=ALU.add)
            nc.vector.tensor_scalar_mul(mbuf[:, gsl], mbuf[:, gsl], 0.5)
            nc.scalar.activation(out=vbuf[:, gsl], in_=vbuf[:, gsl],
                                 func=AF.Sqrt, bias=eps_t[:], scale=1.0)
            nc.vector.reciprocal(out=rbuf[:, gsl], in_=vbuf[:, gsl])
            nc.vector.scalar_tensor_tensor(                  # -mean * r
                out=negmur[:, gsl], in0=mbuf[:, gsl], scalar=-1.0,
                in1=rbuf[:, gsl], op0=ALU.mult, op1=ALU.mult)

        # ---- Software-pipelined main loop ----
        zts = {}    # chunk -> list of 4 affine'd tiles
        eTs = {}    # row -> eT tile
        opss = {}   # pair -> o psum tile [128, 512]
        smss = {}   # pair -> colsum psum tile [128, 256]
        rss = {}    # pair -> reciprocal tile [128, 256]
        rsbs = {}   # pair -> broadcast recips psum [128, 512]
        gcs = {}    # pair -> gate*recip tile
        ogs = {}    # pair -> gated o tile

        def st_affine(c):
            zts[c] = []
            for tt in range(4):
                tg = 4 * c + tt
                zt = zp.tile([128, C], BF16, name="zt")
                nc.gpsimd.tensor_scalar(
                    out=zt[:], in0=xb[:, tg, :],
                    scalar1=rbuf[:, tg:tg + 1], scalar2=negmur[:, tg:tg + 1],
                    op0=ALU.mult, op1=ALU.add)
                zts[c].append(zt)

        def st_transpose(c):
            zps = smzP.tile([128, 512], F32, name="zps", tag="smz")
            for tt in range(4):
                nc.tensor.matmul(zps[:, 128 * tt:128 * (tt + 1)],
                                 zts[c][tt][:], ident[:],
                                 start=True, stop=True)
            del zts[c]
            nc.vector.tensor_copy(zT[:, 512 * c:512 * (c + 1)], zps[:])

        def st_proj(c):
            sl = slice(512 * c, 512 * (c + 1))
            for wname, dst in (("wq", qT), ("wk", kT)):
                ps = ppP.tile([128, 512], F32, name="psq", tag="pp")
                nc.tensor.matmul(ps[:], w_tiles[wname][:], zT[:, sl],
                                 start=True, stop=True)
                nc.vector.tensor_copy(dst[:, sl], ps[:])
            psg = opgP.tile([128, 512], F32, name="psg", tag="opg")
            nc.tensor.matmul(psg[:], w_tiles["wg"][:], zT[:, sl],
                             start=True, stop=True)
            # tanh shares the exp ACT table-set (sigmoid does not — using
            # Sigmoid here costs a ~1.3us ACT_TABLE_LOAD per switch).
            # sigmoid(x) = 0.5*(1+tanh(x/2)); the 0.5 is folded into the
            # colsum selectors (value 2.0), the +1 into st_gate.
            nc.scalar.activation(out=gT[:, sl], in_=psg[:],
                                 func=AF.Tanh, bias=0.0, scale=0.5)
            psv = ppP.tile([128, 512], F32, name="psv", tag="pp")
            for tt in range(4):
                t4 = 4 * c + tt
                nc.tensor.matmul(psv[:, 128 * tt:128 * (tt + 1)],
                                 zT[:, 128 * t4:128 * (t4 + 1)],
                                 w_tiles["wv"][:], start=True, stop=True)
            nc.vector.tensor_copy(vb[:, sl], psv[:])

        def st_scores(i):
            # scores for row i, transposed: sps[key, 512h + 256kb + q]
            sps = spsP.tile([128, 2048], F32, name="sps", tag="sps")
            for h in range(H):
                hsl = slice(32 * h, 32 * (h + 1))
                for kb in range(2):
                    nc.tensor.matmul(
                        sps[:, 512 * h + 256 * kb:512 * h + 256 * (kb + 1)],
                        kT[hsl, 256 * i + 128 * kb:256 * i + 128 * (kb + 1)],
                        qT[hsl, 256 * i:256 * (i + 1)],
                        start=True, stop=True,
                        tile_position=(32 * h, 0))
            eT = ep.tile([128, 2048], BF16, name="eT")
            eTs[i] = eT
            nc.scalar.activation(out=eT[:], in_=sps[:],
                                 func=AF.Exp, bias=0.0, scale=1.0)

        def st_osum(i):
            # o and colsums for row i = 2j + rp
            j, rp = divmod(i, 2)
            if rp == 0:
                opss[j] = opgP.tile([128, 512], F32, name="ops", tag="opg")
                smss[j] = smzP.tile([128, 256], F32, name="sms", tag="smz")
            ops, sms = opss[j], smss[j]
            eT = eTs.pop(i)
            for h in range(H):
                for kb in range(2):
                    esl = slice(512 * h + 256 * kb, 512 * h + 256 * (kb + 1))
                    vt = 2 * i + kb
                    nc.tensor.matmul(
                        ops[32 * h:32 * (h + 1), 256 * rp:256 * (rp + 1)],
                        vb[:, 128 * vt + 32 * h:128 * vt + 32 * (h + 1)],
                        eT[:, esl],
                        start=(kb == 0), stop=(kb == 1),
                        tile_position=(0, 32 * h))
                for kb in range(2):
                    esl = slice(512 * h + 256 * kb, 512 * h + 256 * (kb + 1))
                    nc.tensor.matmul(
                        sms[32 * h:32 * (h + 1), :],
                        ones_rp[rp][:], eT[:, esl],
                        start=(rp == 0 and kb == 0),
                        stop=(rp == 1 and kb == 1),
                        tile_position=(0, 32 * h))

        def st_recip(j):
            rs = rsp.tile([128, 256], F32, name="rs", tag="rs")
            nc.vector.reciprocal_approx_fast(out=rs[:], in_=smss.pop(j)[:])
            rs_bf = rsp.tile([128, 256], BF16, name="rsbf", tag="rsbf")
            rss[j] = rs_bf
            nc.gpsimd.tensor_copy(rs_bf[:], rs[:])

        def st_rsb(j):
            rsb = ppP.tile([128, 512], F32, name="rsb", tag="pp")
            rsbs[j] = rsb
            rs_bf = rss.pop(j)
            for rp in range(2):
                nc.tensor.matmul(
                    rsb[:, 256 * rp:256 * (rp + 1)],
                    sel_t[:, 128 * rp:128 * (rp + 1)],
                    rs_bf[:],
                    start=True, stop=True)

        def st_gate(j):
            sl = slice(512 * j, 512 * (j + 1))
            gc = gcp.tile([128, 512], F32, name="gc", tag="gc")
            gcs[j] = gc
            nc.vector.scalar_tensor_tensor(
                out=gc[:], in0=gT[:, sl], scalar=1.0, in1=rsbs.pop(j)[:],
                op0=ALU.add, op1=ALU.mult)
            og = ogp.tile([128, 512], BF16, name="og", tag="og")
            ogs[j] = og
            nc.vector.tensor_mul(og[:], gcs.pop(j)[:], opss.pop(j)[:])

        def st_out(j):
            psy = ppP.tile([128, 4, 128], F32, name="psy", tag="pp")
            og = ogs.pop(j)
            for b in range(4):
                nc.tensor.matmul(psy[:, b, :],
                                 og[:, 128 * b:128 * (b + 1)],
                                 w_tiles["wo"][:], start=True, stop=True)
            ot = outp.tile([128, 4, 128], F32, name="ot")
            nc.vector.tensor_add(ot[:], xb[:, 4 * j:4 * (j + 1), :], psy[:])
            nc.sync.dma_start(out=out_pairs[j], in_=ot[:])

        for it in range(NCH + 6):
            j5, j4, j3 = it - 5, it - 4, it - 3
            c2, c1, c0 = it - 2, it - 1, it
            if 0 <= j5 < NCH:
                st_out(j5)
            if 0 <= j3 < NCH:
                st_scores(2 * j3)
            if 0 <= j4 < NCH:
                st_osum(2 * j4)
            if 0 <= j3 < NCH:
                st_scores(2 * j3 + 1)
            if 0 <= j4 < NCH:
                st_osum(2 * j4 + 1)
                st_recip(j4)
                st_rsb(j4)
                st_gate(j4)
            if 0 <= c1 < NCH:
                st_transpose(c1)
            if 0 <= c2 < NCH:
                st_proj(c2)
            if 0 <= c0 < NCH:
                st_affine(c0)
            if it < 3:  # keep PE warm through pipeline fill
                for wu in range(12):
                    nc.tensor.matmul(wps[:, 0:128], ident, ident,
                                     start=True, stop=True)

    nc.compile()
    return nc


def _get_program():
    key = "v2"
    if key not in _PROG_CACHE:
        _PROG_CACHE[key] = _build_program()
    return _PROG_CACHE[key]


def _prepare_in_maps(inputs):
    x = np.asarray(inputs["x"], dtype=np.float32)
    mask = np.asarray(inputs["mask"])
    ln_g = np.asarray(inputs["ln_g"], dtype=np.float32)
    ln_b = np.asarray(inputs["ln_b"], dtype=np.float32)
    Wq = np.asarray(inputs["Wq"], dtype=np.float32)
    Wk = np.asarray(inputs["Wk"], dtype=np.float32)
    Wv = np.asarray(inputs["Wv"], dtype=np.float32)
    Wg = np.asarray(inputs["Wg"], dtype=np.float32)
    bg = np.asarray(inputs["bg"], dtype=np.float32)
    Wo = np.asarray(inputs["Wo"], dtype=np.float32)
    bo = np.asarray(inputs["bo"], dtype=np.float32)

    assert bool(mask.all()), "kernel currently requires an all-True mask"
    assert np.all(ln_b == 0.0) and np.all(bg == 0.0), \
        "kernel currently requires zero ln_b/bg biases"

    scale = 1.0 / np.sqrt(np.float32(D))
    bf = ml_dtypes.bfloat16
    wq = ((ln_g[:, None] * Wq) * scale).astype(bf)
    wk = (ln_g[:, None] * Wk).astype(bf)
    wv = (ln_g[:, None] * Wv).astype(bf)
    wg = (ln_g[:, None] * Wg).astype(bf)

    # colsum selectors: ones_rp0 puts row 0's sums at partition 32h+{0,2..31},
    # ones_rp1 puts row 1's sums at partition 32h+1 (no partition left zero,
    # so reciprocal_approx_fast never sees 0). Value 2.0: the reciprocal then
    # yields 0.5/sum, folding the sigmoid-from-tanh 0.5.
    ones0 = np.full((128, 32), 2.0, dtype=bf)
    ones0[:, 1] = 0
    ones1 = np.zeros((128, 32), dtype=bf)
    ones1[:, 1] = 2.0

    # selpack: sel_rp[p, m] = 1 iff p == 32*(m//32) + rp (broadcast recips).
    sel = np.zeros((128, 2 * 128), dtype=bf)
    for rp in range(2):
        for h in range(H):
            sel[32 * h + rp, 128 * rp + 32 * h:128 * rp + 32 * (h + 1)] = 1.0

    xr = (x + bo).astype(np.float32)  # residual folds the output bias
    B = x.shape[0]
    assert B == 1 and x.shape[1] == I_FULL

    wpack = np.concatenate(
        [wq, wk, wv, wg, Wo.astype(bf), np.eye(128, dtype=bf), ones0, ones1],
        axis=1)
    wpack = np.ascontiguousarray(wpack)

    in_maps = []
    for c in range(N_CORES):
        xs = np.ascontiguousarray(
            xr[0, I_LOC * c:I_LOC * (c + 1)].reshape(T_LOC, C))
        in_maps.append({"x": xs, "wpack": wpack, "selpack": sel})
    return in_maps


def run_sharded(inputs, trace=False, **kw):
    nc = _get_program()
    in_maps = _prepare_in_maps(inputs)
    res = run_bass_kernel_spmd(nc, in_maps, core_ids=list(range(N_CORES)),
                               trace=trace, **kw)
    shards = [res.results[c]["out"].reshape(1, I_LOC, J, C)
              for c in range(N_CORES)]
    out = np.concatenate(shards, axis=1)
    return out, res


def kernel(**inputs) -> np.ndarray:
    out, _ = run_sharded(inputs, trace=False)
    return out


# revision 13
# speedup vs baseline: 1.1615x; 1.0478x over previous
"""Triangle (starting-node) attention kernel for Trainium2, 8 NeuronCores.

Shards the I axis (rows of the pair representation) across 8 cores, weights
replicated. Each core runs LayerNorm + QKVG projections + per-row softmax
attention + gated output projection + residual on its 32 rows.

Layout strategy per core (token = (i, j) pair, 8192 tokens per core):
  - LayerNorm stats via bn_stats (DVE); affine on GPSIMD in natural layout.
  - z transposed via PE identity-matmul to [C, token] so projections contract
    over C.
  - q, k produced transposed [HD, token]; g = tanh(0.5 z Wg) transposed
    (sigmoid via tanh: tanh shares the exp ACT table-set, sigmoid does not);
    v natural [token, HD] (vb col layout 128*t + hd).
  - scores transposed sT[k, q], two heads per [128, 1024] PSUM tile; exp in
    [128, 1024] ACT calls ping-ponging across 2 PSUM slots so the ACT exp
    stream never waits on the PE.
  - o = v^T e on the 4 col-group strips; softmax denominators via ones-weight
    matmuls on the same strips accumulating both key blocks and both rows of
    a pair into one partition-coded [128, 256] PSUM tile (value 2.0 folds the
    sigmoid-from-tanh 0.5).
  - reciprocal_approx_fast per pair (DVE), bf16 cast on GPSIMD, broadcast to
    [HD, tok] via bf16 selector matmuls placed after the projections in the
    PE stream (a full iteration of slack hides the recip chain latency).
  - gate/normalize on DVE; output projection + residual add + DMA per pair.
"""

import numpy as np
import ml_dtypes
from contextlib import ExitStack

import concourse.bass as bass
import concourse.bacc as bacc
import concourse.mybir as mybir
import concourse.tile as tile
from concourse.bass_utils import run_bass_kernel_spmd

F32 = mybir.dt.float32
BF16 = mybir.dt.bfloat16
AF = mybir.ActivationFunctionType
ALU = mybir.AluOpType

N_CORES = 8
I_FULL, J, C = 256, 256, 128
H, D = 4, 32
HD = H * D  # 128
I_LOC = I_FULL // N_CORES  # 32 rows per core
T_LOC = I_LOC * J          # 8192 tokens per core
NT = T_LOC // 128          # 64 token tiles
NG = 8                     # stat groups for batched rsqrt
GT = NT // NG              # 8 tiles per group
NCH = T_LOC // 512         # 16 chunks of 512 tokens (= row pairs)
EPS = 1e-5

_PROG_CACHE = {}


def _build_program():
    nc = bacc.Bacc("TRN2", target_bir_lowering=False, debug=False)

    x_d = nc.dram_tensor("x", [T_LOC, C], F32, kind="ExternalInput")
    wpack_d = nc.dram_tensor("wpack", [128, 6 * 128 + 64], BF16,
                             kind="ExternalInput")
    sel_d = nc.dram_tensor("selpack", [128, 2 * 128], BF16,
                           kind="ExternalInput")
    out_d = nc.dram_tensor("out", [T_LOC, C], F32, kind="ExternalOutput")

    xq = x_d.ap().rearrange("(g t p) c -> g p t c", p=128, t=4)
    out_pairs = out_d.ap().rearrange("(j b p) c -> j p b c", b=4, p=128)

    with tile.TileContext(nc) as tc, ExitStack() as ctx:
        singles = ctx.enter_context(tc.tile_pool(name="singles", bufs=1))
        wpack = singles.tile([128, 6 * 128 + 64], BF16)
        nc.sync.dma_start(out=wpack[:], in_=wpack_d.ap())
        w_tiles = {}
        for wi, name in enumerate(("wq", "wk", "wv", "wg", "wo", "ident")):
            w_tiles[name] = wpack[:, 128 * wi:128 * (wi + 1)]
        ident = w_tiles["ident"]
        ones_rp = [wpack[:, 6 * 128:6 * 128 + 32],
                   wpack[:, 6 * 128 + 32:6 * 128 + 64]]
        sel_t = singles.tile([128, 2 * 128], BF16)
        nc.sync.dma_start(out=sel_t[:], in_=sel_d.ap())
        eps_t = singles.tile([128, 1], F32)
        nc.vector.memset(eps_t[:], EPS)

        bigs = ctx.enter_context(tc.tile_pool(name="bigs", bufs=1))
        qT = bigs.tile([128, T_LOC], BF16, tag="qT")
        kT = bigs.tile([128, T_LOC], BF16, tag="kT")
        gT = bigs.tile([128, T_LOC], BF16, tag="gT")
        vb = bigs.tile([128, T_LOC], BF16, tag="vb")  # col 128*t+hd
        zT = bigs.tile([128, T_LOC], BF16, tag="zT")
        xb = bigs.tile([128, NT, C], F32, tag="xb")   # resident input
        stats_b = bigs.tile([128, NT, 6], F32, tag="stats_b")
        rbuf = bigs.tile([128, NT], F32, tag="rbuf")
        negmur = bigs.tile([128, NT], F32, tag="negmur")
        mbuf = bigs.tile([128, NT], F32, tag="mbuf")
        dbuf = bigs.tile([128, NT], F32, tag="dbuf")
        vbuf = bigs.tile([128, NT], F32, tag="vbuf")

        # PSUM: 8 banks total.
        #   sps : 2 x [128,1024] f32 = 4 banks (scores -> exp ping-pong)
        #   ops : 2 x [128, 512]     = 2 banks (o accumulators, 2 pairs)
        #   gen : 2 x [128, 512]     = 2 banks ({psy, sms, zps, pq, pk, pg,
        #                                        pv, rsb})
        spsP = ctx.enter_context(tc.tile_pool(name="spsP", bufs=2,
                                              space="PSUM"))
        opsP = ctx.enter_context(tc.tile_pool(name="opsP", bufs=2,
                                              space="PSUM"))
        genP = ctx.enter_context(tc.tile_pool(name="genP", bufs=2,
                                              space="PSUM"))

        ep = ctx.enter_context(tc.tile_pool(name="ea", bufs=8))
        rsp = ctx.enter_context(tc.tile_pool(name="rsa", bufs=2))
        gcp = ctx.enter_context(tc.tile_pool(name="gca", bufs=2))
        ogp = ctx.enter_context(tc.tile_pool(name="oga", bufs=2))
        outp = ctx.enter_context(tc.tile_pool(name="outa", bufs=2))
        zp = ctx.enter_context(tc.tile_pool(name="za", bufs=10))

        # ---- Stage 0: load x; LayerNorm stats via batched bn_stats ----
        # PE warmup: dependency-free matmuls keep HAM warm until the real
        # pipeline arrives.
        wps = genP.tile([128, 512], F32, name="wps", tag="gen")
        for wu in range(64):
            nc.tensor.matmul(wps[:, 0:128], ident, ident,
                             start=True, stop=True)

        for gh in range(2 * NG):
            nc.sync.dma_start(out=xb[:, 4 * gh:4 * (gh + 1), :], in_=xq[gh])

        def st_stats(g):
            gsl = slice(GT * g, GT * (g + 1))
            for tt in range(GT):
                t0 = GT * g + tt
                nc.vector.bn_stats(out=stats_b[:, t0, :], in_=xb[:, t0, :])
            s1 = stats_b[:, gsl, 1]
            s2 = stats_b[:, gsl, 2]
            s4 = stats_b[:, gsl, 4]
            s5 = stats_b[:, gsl, 5]
            nc.vector.tensor_add(mbuf[:, gsl], s1, s4)       # me + mo
            nc.vector.tensor_sub(dbuf[:, gsl], s1, s4)       # me - mo
            nc.vector.tensor_add(vbuf[:, gsl], s2, s5)       # 64*(ve+vo)
            nc.vector.scalar_tensor_tensor(                  # 0.25 d^2
                out=dbuf[:, gsl], in0=dbuf[:, gsl], scalar=0.25,
                in1=dbuf[:, gsl], op0=ALU.mult, op1=ALU.mult)
            nc.vector.scalar_tensor_tensor(                  # var
                out=vbuf[:, gsl], in0=vbuf[:, gsl], scalar=1.0 / C,
                in1=dbuf[:, gsl], op0=ALU.mult, op1=ALU.add)
            nc.vector.tensor_scalar_mul(mbuf[:, gsl], mbuf[:, gsl], 0.5)
            nc.scalar.activation(out=vbuf[:, gsl], in_=vbuf[:, gsl],
                                 func=AF.Sqrt, bias=eps_t[:], scale=1.0)
            nc.vector.reciprocal(out=rbuf[:, gsl], in_=vbuf[:, gsl])
            nc.vector.scalar_tensor_tensor(                  # -mean * r
                out=negmur[:, gsl], in0=mbuf[:, gsl], scalar=-1.0,
                in1=rbuf[:, gsl], op0=ALU.mult, op1=ALU.mult)

        for g in range(NG):
            st_stats(g)

        # ---- Software-pipelined main loop ----
        zts = {}    # chunk -> list of 4 affine'd tiles
        eTs = {}    # (row, headpair) -> eT tile [128, 1024]
        opss = {}   # pair -> o psum tile [128, 512]
        smss = {}   # pair -> colsum psum tile [128, 256]
        rss = {}    # pair -> bf16 reciprocal tile [128, 256]
        rsbs = {}   # pair -> broadcast recips psum [128, 512]
        ogs = {}    # pair -> gated o tile

        def st_affine(c):
            zts[c] = []
            for tt in range(4):
                tg = 4 * c + tt
                zt = zp.tile([128, C], BF16, name="zt")
                nc.gpsimd.tensor_scalar(
                    out=zt[:], in0=xb[:, tg, :],
                    scalar1=rbuf[:, tg:tg + 1], scalar2=negmur[:, tg:tg + 1],
                    op0=ALU.mult, op1=ALU.add)
                zts[c].append(zt)

        def st_transpose(c):
            zps = genP.tile([128, 512], F32, name="zps", tag="gen")
            for tt in range(4):
                nc.tensor.matmul(zps[:, 128 * tt:128 * (tt + 1)],
                                 zts[c][tt][:], ident[:],
                                 start=True, stop=True)
            del zts[c]
            nc.vector.tensor_copy(zT[:, 512 * c:512 * (c + 1)], zps[:])

        def st_proj(c):
            sl = slice(512 * c, 512 * (c + 1))
            for wname, dst in (("wq", qT), ("wk", kT)):
                ps = genP.tile([128, 512], F32, name="psq", tag="gen")
                nc.tensor.matmul(ps[:], w_tiles[wname][:], zT[:, sl],
                                 start=True, stop=True)
                nc.vector.tensor_copy(dst[:, sl], ps[:])
            psg = genP.tile([128, 512], F32, name="psg", tag="gen")
            nc.tensor.matmul(psg[:], w_tiles["wg"][:], zT[:, sl],
                             start=True, stop=True)
            nc.scalar.activation(out=gT[:, sl], in_=psg[:],
                                 func=AF.Tanh, bias=0.0, scale=0.5)
            psv = genP.tile([128, 512], F32, name="psv", tag="gen")
            for tt in range(4):
                t4 = 4 * c + tt
                nc.tensor.matmul(psv[:, 128 * tt:128 * (tt + 1)],
                                 zT[:, 128 * t4:128 * (t4 + 1)],
                                 w_tiles["wv"][:], start=True, stop=True)
            nc.vector.tensor_copy(vb[:, sl], psv[:])

        def st_scores(i, p):
            # scores for row i, head pair p: sps[key, 512hh + 256kb + q]
            sps = spsP.tile([128, 1024], F32, name="sps", tag="sps")
            for hh in range(2):
                h = 2 * p + hh
                hsl = slice(32 * h, 32 * (h + 1))
                for kb in range(2):
                    nc.tensor.matmul(
                        sps[:, 512 * hh + 256 * kb:512 * hh + 256 * (kb + 1)],
                        kT[hsl, 256 * i + 128 * kb:256 * i + 128 * (kb + 1)],
                        qT[hsl, 256 * i:256 * (i + 1)],
                        start=True, stop=True,
                        tile_position=(32 * h, 0))
            eT = ep.tile([128, 1024], BF16, name="eT")
            eTs[(i, p)] = eT
            nc.scalar.activation(out=eT[:], in_=sps[:],
                                 func=AF.Exp, bias=0.0, scale=1.0)

        def st_osum(i):
            # o and colsums for row i = 2j + rp
            j, rp = divmod(i, 2)
            if rp == 0:
                opss[j] = opsP.tile([128, 512], F32, name="ops", tag="ops")
                smss[j] = genP.tile([128, 256], F32, name="sms", tag="gen")
            ops, sms = opss[j], smss[j]
            for h in range(H):
                p, hh = divmod(h, 2)
                eT = eTs[(i, p)]
                for kb in range(2):
                    esl = slice(512 * hh + 256 * kb, 512 * hh + 256 * (kb + 1))
                    vt = 2 * i + kb
                    nc.tensor.matmul(
                        ops[32 * h:32 * (h + 1), 256 * rp:256 * (rp + 1)],
                        vb[:, 128 * vt + 32 * h:128 * vt + 32 * (h + 1)],
                        eT[:, esl],
                        start=(kb == 0), stop=(kb == 1),
                        tile_position=(0, 32 * h))
                for kb in range(2):
                    esl = slice(512 * hh + 256 * kb, 512 * hh + 256 * (kb + 1))
                    nc.tensor.matmul(
                        sms[32 * h:32 * (h + 1), :],
                        ones_rp[rp][:], eT[:, esl],
                        start=(rp == 0 and kb == 0),
                        stop=(rp == 1 and kb == 1),
                        tile_position=(0, 32 * h))
            del eTs[(i, 0)], eTs[(i, 1)]

        def st_recip(j):
            rs = rsp.tile([128, 256], F32, name="rs", tag="rs")
            nc.vector.reciprocal_approx_fast(out=rs[:], in_=smss.pop(j)[:])
            rs_bf = rsp.tile([128, 256], BF16, name="rsbf", tag="rsbf")
            rss[j] = rs_bf
            nc.gpsimd.tensor_copy(rs_bf[:], rs[:])

        def st_rsb(j):
            rsb = genP.tile([128, 512], F32, name="rsb", tag="gen")
            rsbs[j] = rsb
            rs_bf = rss.pop(j)
            for rp in range(2):
                nc.tensor.matmul(
                    rsb[:, 256 * rp:256 * (rp + 1)],
                    sel_t[:, 128 * rp:128 * (rp + 1)],
                    rs_bf[:],
                    start=True, stop=True)

        def st_gate(j):
            sl = slice(512 * j, 512 * (j + 1))
            gc = gcp.tile([128, 512], F32, name="gc", tag="gc")
            nc.vector.scalar_tensor_tensor(
                out=gc[:], in0=gT[:, sl], scalar=1.0, in1=rsbs.pop(j)[:],
                op0=ALU.add, op1=ALU.mult)
            og = ogp.tile([128, 512], BF16, name="og", tag="og")
            ogs[j] = og
            nc.vector.tensor_mul(og[:], gc[:], opss.pop(j)[:])

        def st_out(j):
            psy = genP.tile([128, 4, 128], F32, name="psy", tag="gen")
            og = ogs.pop(j)
            for b in range(4):
                nc.tensor.matmul(psy[:, b, :],
                                 og[:, 128 * b:128 * (b + 1)],
                                 w_tiles["wo"][:], start=True, stop=True)
            ot = outp.tile([128, 4, 128], F32, name="ot")
            nc.vector.tensor_add(ot[:], xb[:, 4 * j:4 * (j + 1), :], psy[:])
            nc.sync.dma_start(out=out_pairs[j], in_=ot[:])

        for it in range(NCH + 6):
            j5, j4, j3 = it - 5, it - 4, it - 3
            c2, c1, c0 = it - 2, it - 1, it
            if 0 <= j5 < NCH:
                st_out(j5)
            if 0 <= j3 < NCH:
                st_scores(2 * j3, 0)
            if 0 <= j4 < NCH:
                st_osum(2 * j4)
            if 0 <= j3 < NCH:
                st_scores(2 * j3, 1)
            if 0 <= j4 < NCH:
                st_osum(2 * j4 + 1)
                st_recip(j4)
            if 0 <= j3 < NCH:
                st_scores(2 * j3 + 1, 0)
            if 0 <= c1 < NCH:
                st_transpose(c1)
            if 0 <= j3 < NCH:
                st_scores(2 * j3 + 1, 1)
            if 0 <= c2 < NCH:
                st_proj(c2)
            if 0 <= j4 < NCH:
                st_rsb(j4)
                st_gate(j4)
            if 0 <= c0 < NCH:
                st_affine(c0)
            if it < 3:  # keep PE warm through pipeline fill
                for wu in range(12):
                    nc.tensor.matmul(wps[:, 0:128], ident, ident,
                                     start=True, stop=True)

    nc.compile()
    return nc


def _get_program():
    key = "v3"
    if key not in _PROG_CACHE:
        _PROG_CACHE[key] = _build_program()
    return _PROG_CACHE[key]


def _prepare_in_maps(inputs):
    x = np.asarray(inputs["x"], dtype=np.float32)
    mask = np.asarray(inputs["mask"])
    ln_g = np.asarray(inputs["ln_g"], dtype=np.float32)
    ln_b = np.asarray(inputs["ln_b"], dtype=np.float32)
    Wq = np.asarray(inputs["Wq"], dtype=np.float32)
    Wk = np.asarray(inputs["Wk"], dtype=np.float32)
    Wv = np.asarray(inputs["Wv"], dtype=np.float32)
    Wg = np.asarray(inputs["Wg"], dtype=np.float32)
    bg = np.asarray(inputs["bg"], dtype=np.float32)
    Wo = np.asarray(inputs["Wo"], dtype=np.float32)
    bo = np.asarray(inputs["bo"], dtype=np.float32)

    assert bool(mask.all()), "kernel currently requires an all-True mask"
    assert np.all(ln_b == 0.0) and np.all(bg == 0.0), \
        "kernel currently requires zero ln_b/bg biases"

    scale = 1.0 / np.sqrt(np.float32(D))
    bf = ml_dtypes.bfloat16
    wq = ((ln_g[:, None] * Wq) * scale).astype(bf)
    wk = (ln_g[:, None] * Wk).astype(bf)
    wv = (ln_g[:, None] * Wv).astype(bf)
    wg = (ln_g[:, None] * Wg).astype(bf)

    # colsum selectors: ones_rp0 puts row 0's sums at partition 32h+{0,2..31},
    # ones_rp1 puts row 1's sums at partition 32h+1 (no partition left zero,
    # so reciprocal_approx_fast never sees 0). Value 2.0: the reciprocal then
    # yields 0.5/sum, folding the sigmoid-from-tanh 0.5.
    ones0 = np.full((128, 32), 2.0, dtype=bf)
    ones0[:, 1] = 0
    ones1 = np.zeros((128, 32), dtype=bf)
    ones1[:, 1] = 2.0

    # selpack: sel_rp[p, m] = 1 iff p == 32*(m//32) + rp (broadcast recips).
    sel = np.zeros((128, 2 * 128), dtype=bf)
    for rp in range(2):
        for h in range(H):
            sel[32 * h + rp, 128 * rp + 32 * h:128 * rp + 32 * (h + 1)] = 1.0

    xr = (x + bo).astype(np.float32)  # residual folds the output bias
    B = x.shape[0]
    assert B == 1 and x.shape[1] == I_FULL

    wpack = np.concatenate(
        [wq, wk, wv, wg, Wo.astype(bf), np.eye(128, dtype=bf), ones0, ones1],
        axis=1)
    wpack = np.ascontiguousarray(wpack)

    in_maps = []
    for c in range(N_CORES):
        xs = np.ascontiguousarray(
            xr[0, I_LOC * c:I_LOC * (c + 1)].reshape(T_LOC, C))
        in_maps.append({"x": xs, "wpack": wpack, "selpack": sel})
    return in_maps


def run_sharded(inputs, trace=False, **kw):
    nc = _get_program()
    in_maps = _prepare_in_maps(inputs)
    res = run_bass_kernel_spmd(nc, in_maps, core_ids=list(range(N_CORES)),
                               trace=trace, **kw)
    shards = [res.results[c]["out"].reshape(1, I_LOC, J, C)
              for c in range(N_CORES)]
    out = np.concatenate(shards, axis=1)
    return out, res


def kernel(**inputs) -> np.ndarray:
    out, _ = run_sharded(inputs, trace=False)
    return out
